# revision 36
# baseline (speedup 1.0000x reference)
"""AttentionPooling (segment_reduce) Trainium2 kernel.

att = sigmoid([input_rep, final_rep] @ W_lin.T + b_lin)
g   = att * (final_rep @ W_last.T + b_last)
out = segment_sum(g, graph_index, 16384)          # graph_index sorted

Strategy (8 NeuronCores, pure data-parallel, no collectives):
  graph_index is sorted, so a contiguous node range covers a contiguous
  graph range.  Host greedily packs whole graphs into "windows" of
  <= WIN_NODES nodes spanning <= 128 graphs; ~136 windows cover all 500k
  nodes = 8 cores x 17 windows.  Each core gets its windows as a padded
  node stream in feature-major bf16 layout (host pre-transposes + casts,
  so the device only does line-rate DMA and matmuls).

Device pipeline (flat over all subtiles of 128 nodes, paired 2-by-2):
  PE    : per subtile  ones[1,128].T @ biascat  (psum init, row-packed)
          xT_in.T @ WlinT[:128] / xT_fin0.T @ [WlinT[128:256]|WlastT[:128]]
          / xT_fin1.T @ [WlinT[256:]|WlastT[128:]]  -> psum half [128,512]
          (pairs of subtiles share one 2-bank psum tile [128,1024])
  ACT   : sigmoid over BOTH halves' att cols in one 512-col strided op
  DVE   : g = att * val-cols, one 512-col strided op per pair
  PE    : onehot.T @ g accumulated into a window's seg psum, emitted with
          a 4-subtile LAG so the strict-FIFO PE never waits on ACT/DVE.
          onehots are host-precomputed and DMA'd as a 4th bf16 plane
          (device-built onehots cost 280ns/subtile of DVE or 2.4us of
          GpSimd — both measured worse than the extra DMA).
  Per window the [128,256] f32 graph block is copied out + DMA'd; host
  reassembles the window blocks into [16384, 256].
"""

import numpy as np
import ml_dtypes

import concourse.bacc as bacc
import concourse.tile as tile
from concourse import mybir
from concourse import bass_utils
from concourse._compat import with_exitstack

P = 128
HID = 256
WIN_SUB = 29                     # subtiles (128 nodes) per window
WIN_NODES = WIN_SUB * P          # 3712
WINDOWS_PER_CORE = 17
N_CORES = 8
NUM_GRAPHS = 16384
GMAX = P                         # graph span per window
LAG = 6                          # subtiles between linears and seg matmul
                                 # (ACT+DVE chain after psum-stop is ~2us;
                                 # LAG=4 measured 1.16us stalls on seg MMs)

BF16 = mybir.dt.bfloat16
F32 = mybir.dt.float32
F8 = mybir.dt.float8e4
npbf16 = ml_dtypes.bfloat16
npf8 = ml_dtypes.float8_e4m3


# ----------------------------------------------------------------------------
# host-side planning
# ----------------------------------------------------------------------------

def _build_windows(gi: np.ndarray, num_graphs: int):
    """Greedy windows: contiguous whole-graph ranges, graph span <= GMAX,
    node count <= WIN_NODES.  Returns list of (gbase, gcnt, nstart, ncnt)."""
    counts = np.bincount(gi, minlength=num_graphs)
    starts = np.concatenate([[0], np.cumsum(counts)])
    wins = []
    g = 0
    while g < num_graphs:
        base = g
        nodes = 0
        cnt = 0
        while g < num_graphs and cnt < GMAX and nodes + counts[g] <= WIN_NODES:
            nodes += int(counts[g])
            cnt += 1
            g += 1
        if cnt == 0:
            raise ValueError(f"graph {g} has {counts[g]} nodes > {WIN_NODES}")
        wins.append((base, cnt, int(starts[base]), nodes))
    return wins


# ----------------------------------------------------------------------------
# device kernel
# ----------------------------------------------------------------------------

@with_exitstack
def _device_kernel(ctx, tc, out_ap, ins, n_windows):
    nc = tc.nc
    xdr_ap, xf0_ap, xf1_ap, oh_ap, wdr_ap, wlast0_ap, wcat1_ap, \
        biascat_ap, ones_ap = ins

    consts = ctx.enter_context(tc.tile_pool(name="consts", bufs=1))
    xpool = ctx.enter_context(tc.tile_pool(name="x", bufs=3))
    x0pool = ctx.enter_context(tc.tile_pool(name="x0", bufs=1))
    apool = ctx.enter_context(tc.tile_pool(name="act", bufs=3))
    gpool = ctx.enter_context(tc.tile_pool(name="g", bufs=5))
    outpool = ctx.enter_context(tc.tile_pool(name="out", bufs=2))
    ps_pair = ctx.enter_context(tc.tile_pool(name="ps_pair", bufs=3, space="PSUM"))
    ps_seg = ctx.enter_context(tc.tile_pool(name="ps_seg", bufs=2, space="PSUM"))

    # constants, loaded once; ordered so the first matmuls' inputs land first
    ones_t = consts.tile([P, P], BF16)
    nc.sync.dma_start(ones_t[:], ones_ap[:])
    biascat = consts.tile([P, 2 * HID], BF16)
    nc.sync.dma_start(biascat[:], biascat_ap[:])
    wdr = consts.tile([P, 2 * HID], F8)
    nc.sync.dma_start(wdr[:], wdr_ap[:])
    wlast0 = consts.tile([P, HID], BF16)
    nc.sync.dma_start(wlast0[:], wlast0_ap[:])
    wcat1 = consts.tile([P, 2 * HID], BF16)
    nc.sync.dma_start(wcat1[:], wcat1_ap[:])
    wdr3 = wdr.rearrange("p (a b) -> p a b", a=2)

    T = n_windows * WIN_SUB

    # ---- per-window x tiles (window 0 staged in chunks so matmuls can
    # start after ~0.3 MB; later windows are one bulk DMA per plane) ----
    NCHUNK = 3
    win_tiles = [None] * n_windows

    def load_window(w):
        base = w * WIN_NODES
        # xdr plane is window-major [xin8_w | xf08_w] fp8, 2*WIN_NODES cols/window
        db = 2 * base
        if w == 0:
            tiles = []
            # leading chunks shrink toward the front (2,2,4,4 subtiles) so the
            # first matmuls start after ~0.15 MB of DMA instead of ~0.3 MB
            bounds = [0, 2 * P, 4 * P, 8 * P, 12 * P]
            for q in range(len(bounds) - 1):
                c0 = bounds[q]
                cw = bounds[q + 1] - c0
                xt = x0pool.tile([P, 2 * cw], F8, tag=f"x0i{q}")
                nc.sync.dma_start(xt[:, 0:cw], xdr_ap[:, db + c0:db + c0 + cw])
                nc.sync.dma_start(
                    xt[:, cw:2 * cw],
                    xdr_ap[:, db + WIN_NODES + c0:db + WIN_NODES + c0 + cw])
                f0 = x0pool.tile([P, cw], BF16, tag=f"x0a{q}")
                nc.sync.dma_start(f0[:], xf0_ap[:, base + c0:base + c0 + cw])
                f1 = x0pool.tile([P, cw], BF16, tag=f"x0b{q}")
                nc.sync.dma_start(f1[:], xf1_ap[:, base + c0:base + c0 + cw])
                oht = x0pool.tile([P, cw], BF16, tag=f"x0o{q}")
                nc.sync.dma_start(oht[:], oh_ap[:, base + c0:base + c0 + cw])
                tiles.append((xt, f0, f1, oht, c0, c0 + cw))
            r0 = bounds[-1]
            rw = WIN_NODES - r0
            xr = x0pool.tile([P, 2 * rw], F8, tag="x0ir")
            nc.sync.dma_start(xr[:, 0:rw], xdr_ap[:, db + r0:db + WIN_NODES])
            nc.sync.dma_start(
                xr[:, rw:2 * rw],
                xdr_ap[:, db + WIN_NODES + r0:db + 2 * WIN_NODES])
            f0r = x0pool.tile([P, rw], BF16, tag="x0ar")
            nc.sync.dma_start(f0r[:], xf0_ap[:, base + r0:base + WIN_NODES])
            f1r = x0pool.tile([P, rw], BF16, tag="x0br")
            nc.sync.dma_start(f1r[:], xf1_ap[:, base + r0:base + WIN_NODES])
            ohr = x0pool.tile([P, rw], BF16, tag="x0or")
            nc.sync.dma_start(ohr[:], oh_ap[:, base + r0:base + WIN_NODES])
            tiles.append((xr, f0r, f1r, ohr, r0, WIN_NODES))
            win_tiles[w] = tiles
        else:
            xdr_t = xpool.tile([P, 2 * WIN_NODES], F8, tag="xdr")
            nc.sync.dma_start(xdr_t[:], xdr_ap[:, db:db + 2 * WIN_NODES])
            xf0_t = xpool.tile([P, WIN_NODES], BF16, tag="xf0")
            nc.sync.dma_start(xf0_t[:], xf0_ap[:, base:base + WIN_NODES])
            xf1_t = xpool.tile([P, WIN_NODES], BF16, tag="xf1")
            nc.sync.dma_start(xf1_t[:], xf1_ap[:, base:base + WIN_NODES])
            oh_t = xpool.tile([P, WIN_NODES], BF16, tag="oh")
            nc.sync.dma_start(oh_t[:], oh_ap[:, base:base + WIN_NODES])
            win_tiles[w] = [(xdr_t, xf0_t, xf1_t, oh_t, 0, WIN_NODES)]

    def subtile_x(t):
        """-> (xt, f0, f1, oht, col) for global subtile t."""
        w, s = divmod(t, WIN_SUB)
        c = s * P
        for xt, f0, f1, oht, lo, hi in win_tiles[w]:
            if lo <= c < hi:
                return xt, f0, f1, oht, c - lo
        raise AssertionError

    def emit_mms(ps, off, t):
        """3 accumulating matmuls for subtile t into psum cols [off, off+512).
        The att head (xin + first final half vs WlinT[0:256]) runs as one fp8
        DoubleRow matmul (K=256 virtualized, 2 cols/cycle); the rest is bf16."""
        xdr, f0, f1, _, col = subtile_x(t)
        xdr3 = xdr.rearrange("p (a b) -> p a b", a=2)
        nc.tensor.matmul(ps[:, off:off + HID],
                         lhsT=xdr3[:, :, col:col + P], rhs=wdr3,
                         start=False, stop=False,
                         perf_mode=mybir.MatmulPerfMode.DoubleRow)
        nc.tensor.matmul(ps[:, off + HID:off + 2 * HID],
                         lhsT=f0[:, col:col + P],
                         rhs=wlast0[:, :], start=False, stop=False)
        nc.tensor.matmul(ps[:, off:off + 2 * HID], lhsT=f1[:, col:col + P],
                         rhs=wcat1[:, :], start=False, stop=True)

    def emit_bias(ps, off, row):
        # K=32 all-ones matmul against a biascat whose rows are bias at
        # k%32==0 and zero elsewhere: each of the 4 group members targets a
        # distinct 32-row strip + distinct PSUM bank so they overlap in the
        # array — one 512-col span initializes four subtile psum halves.
        nc.tensor.matmul(ps[:, off:off + 2 * HID],
                         lhsT=ones_t[32 * row:32 * (row + 1), 0:P],
                         rhs=biascat[32 * row:32 * (row + 1), :],
                         start=True, stop=False,
                         tile_position=(32 * row, 0))

    # ---- flat software pipeline over all subtiles ----
    # state rings
    ps_of = {}           # pair index -> psum tile
    g_of = {}            # pair index -> g pair tile [P, 512] (bf16)
    seg_of = {}          # window -> seg psum tile

    n_pairs = T // 2     # T odd: last subtile handled unpaired
    assert T % 2 == 1

    def emit_pair_post(p):
        """ACT sigmoid + DVE mult for pair p (subtiles 2p, 2p+1), each as one
        512-col strided op over both psum halves."""
        ps2 = ps_of[p].rearrange("p (a b) -> p a b", a=2)
        att = apool.tile([P, 2 * HID], BF16, tag="att")
        att2 = att.rearrange("p (a b) -> p a b", a=2)
        nc.scalar.activation(att2, ps2[:, :, 0:HID],
                             mybir.ActivationFunctionType.Sigmoid)
        g_sb = gpool.tile([P, 2 * HID], BF16, tag="g")
        nc.vector.tensor_tensor(
            g_sb.rearrange("p (a b) -> p a b", a=2), att2,
            ps2[:, :, HID:2 * HID], op=mybir.AluOpType.mult)
        g_of[p] = g_sb

    def emit_seg(u):
        """seg matmul for subtile u (called LAG subtiles later)."""
        w, s = divmod(u, WIN_SUB)
        if s == 0:
            # padded to a full 2KB bank so the two in-flight seg windows
            # never share a bank (PE-W + ACT-R same bank is fatal)
            seg_of[w] = ps_seg.tile([P, 2 * HID], F32, tag="seg", name="seg")
        seg = seg_of[w]
        g_sb = g_of[u // 2]
        half = u % 2
        _, _, _, oht, col = subtile_x(u)
        nc.tensor.matmul(seg[:, 0:HID], lhsT=oht[:, col:col + P],
                         rhs=g_sb[:, half * HID:(half + 1) * HID],
                         start=(s == 0), stop=(s == WIN_SUB - 1))
        if s == WIN_SUB - 1:
            out_t = outpool.tile([P, HID], F32)
            nc.scalar.copy(out_t[:], seg[:, 0:HID])
            nc.sync.dma_start(out_ap[w * P:(w + 1) * P, :], out_t[:])

    load_window(0)
    for t in range(T):
        w, s = divmod(t, WIN_SUB)
        if s == 0:
            if w == 0 and n_windows > 1:
                load_window(1)
            if w + 2 < n_windows:
                load_window(w + 2)
        if t % 4 == 0:
            # 4 bias matmuls (2 pair psum tiles) per group
            for u in range(t, min(t + 4, T)):
                if u % 2 == 0:
                    ps_of[u // 2] = ps_pair.tile([P, 4 * HID], F32, tag="ps", name="ps")
                emit_bias(ps_of[u // 2], (u % 2) * 2 * HID, u % 4)
        if t == T - 1:
            # unpaired last subtile (psum + bias already emitted by its group)
            ps = ps_of[t // 2]
            emit_mms(ps, 0, t)
            att1 = apool.tile([P, HID], BF16, tag="att1")
            nc.scalar.activation(att1[:], ps[:, 0:HID],
                                 mybir.ActivationFunctionType.Sigmoid)
            g1 = gpool.tile([P, 2 * HID], BF16, tag="g")
            nc.vector.tensor_tensor(g1[:, 0:HID], att1[:], ps[:, HID:2 * HID],
                                    op=mybir.AluOpType.mult)
            g_of[t // 2] = g1
        else:
            emit_mms(ps_of[t // 2], (t % 2) * 2 * HID, t)
            if t % 2 == 1:
                emit_pair_post(t // 2)
        if t >= LAG:
            emit_seg(t - LAG)
    for u in range(T - LAG, T):
        emit_seg(u)


def build_module(n_windows=WINDOWS_PER_CORE):
    nc = bacc.Bacc("TRN2", debug=False, num_devices=N_CORES)
    nn = n_windows * WIN_NODES
    ins = [
        nc.dram_tensor("xdr", [P, 2 * nn], F8, kind="ExternalInput").ap(),
        nc.dram_tensor("xf0", [P, nn], BF16, kind="ExternalInput").ap(),
        nc.dram_tensor("xf1", [P, nn], BF16, kind="ExternalInput").ap(),
        nc.dram_tensor("oh", [P, nn], BF16, kind="ExternalInput").ap(),
        nc.dram_tensor("wdr", [P, 2 * HID], F8, kind="ExternalInput").ap(),
        nc.dram_tensor("wlast0", [P, HID], BF16, kind="ExternalInput").ap(),
        nc.dram_tensor("wcat1", [P, 2 * HID], BF16, kind="ExternalInput").ap(),
        nc.dram_tensor("biascat", [P, 2 * HID], BF16, kind="ExternalInput").ap(),
        nc.dram_tensor("ones", [P, P], BF16, kind="ExternalInput").ap(),
    ]
    out_ap = nc.dram_tensor("out", [n_windows * P, HID], F32,
                            kind="ExternalOutput").ap()
    with tile.TileContext(nc) as tc:
        _device_kernel(tc, out_ap, ins, n_windows)
    nc.compile()
    return nc


# ----------------------------------------------------------------------------
# host-side data prep
# ----------------------------------------------------------------------------

def _prep(inputs, n_windows):
    gi = np.asarray(inputs["graph_index"]).astype(np.int64)
    x_in = np.asarray(inputs["input_rep"], dtype=np.float32)
    x_fin = np.asarray(inputs["final_rep"], dtype=np.float32)
    W_lin = np.asarray(inputs["W_lin"], dtype=np.float32)
    b_lin = np.asarray(inputs["b_lin"], dtype=np.float32)
    W_last = np.asarray(inputs["W_last"], dtype=np.float32)
    b_last = np.asarray(inputs["b_last"], dtype=np.float32)

    if np.any(np.diff(gi) < 0):
        order = np.argsort(gi, kind="stable")
        gi = gi[order]
        x_in = x_in[order]
        x_fin = x_fin[order]

    wins = _build_windows(gi, NUM_GRAPHS)
    budget = N_CORES * n_windows
    assert len(wins) <= budget, f"{len(wins)} windows > budget {budget}"
    wins = wins + [(NUM_GRAPHS, 0, len(gi), 0)] * (budget - len(wins))

    x_in8 = x_in.astype(npf8)                 # att head via fp8 DoubleRow
    x_f08 = np.ascontiguousarray(x_fin[:, 0:P]).astype(npf8)
    x_fin_b = x_fin.astype(npbf16)

    WlinT = W_lin.T                           # [384, 256] f32
    WlastT = W_last.T.astype(npbf16)          # [256, 256]
    wdr = np.concatenate(
        [WlinT[0:P].astype(npf8), WlinT[P:2 * P].astype(npf8)], axis=1)
    wlast0 = np.ascontiguousarray(WlastT[0:P])
    wcat1 = np.ascontiguousarray(
        np.concatenate([WlinT[2 * P:3 * P].astype(npbf16),
                        WlastT[P:2 * P]], axis=1))
    # bias rows at k%32==0 only: the K=32 all-ones bias matmul reduces the
    # 32-row strip, so a single bias row per strip survives
    biascat = np.zeros((P, 2 * HID), npbf16)
    biascat[0::32, :] = np.concatenate([b_lin, b_last]).astype(npbf16)
    ones_t = np.ones((P, P), npbf16)
    jrange = np.arange(P, dtype=np.int32)

    nn = n_windows * WIN_NODES
    in_maps = []
    for c in range(N_CORES):
        xdr_p = np.zeros((P, 2 * nn), npf8)
        xf0_p = np.zeros((P, nn), npbf16)
        xf1_p = np.zeros((P, nn), npbf16)
        oh_p = np.zeros((P, nn), npbf16)
        for j in range(n_windows):
            gb, gc, ns, ncnt = wins[c * n_windows + j]
            if ncnt == 0:
                continue
            off = j * WIN_NODES
            xdr_p[:, 2 * off:2 * off + ncnt] = x_in8[ns:ns + ncnt].T
            xdr_p[:, 2 * off + WIN_NODES:2 * off + WIN_NODES + ncnt] = \
                x_f08[ns:ns + ncnt].T
            xf0_p[:, off:off + ncnt] = x_fin_b[ns:ns + ncnt, 0:P].T
            xf1_p[:, off:off + ncnt] = x_fin_b[ns:ns + ncnt, P:2 * P].T
            # onehot plane: subtile s = cols [s*128,(s+1)*128); oh[p, s*128+j]
            # = 1 iff node (s*128+p) of the window belongs to local graph j
            flat = np.full((WIN_NODES,), -1, np.int32)
            flat[0:ncnt] = (gi[ns:ns + ncnt] - gb).astype(np.int32)
            idx_mat = flat.reshape(WIN_SUB, P)            # [s, p]
            oh_win = (idx_mat[:, :, None] == jrange).astype(npbf16)  # [s,p,j]
            oh_p[:, off:off + WIN_NODES] = \
                oh_win.transpose(1, 0, 2).reshape(P, WIN_NODES)
        in_maps.append({
            "xdr": xdr_p, "xf0": xf0_p, "xf1": xf1_p, "oh": oh_p,
            "wdr": wdr, "wlast0": wlast0, "wcat1": wcat1,
            "biascat": biascat, "ones": ones_t,
        })
    return wins, in_maps


def _assemble(wins, results, n_windows):
    out = np.zeros((NUM_GRAPHS, HID), np.float32)
    for c in range(N_CORES):
        res = results[c]["out"]
        for j in range(n_windows):
            gb, gc, _, _ = wins[c * n_windows + j]
            if gc == 0:
                continue
            out[gb:gb + gc] = res[j * P:j * P + gc]
    return out


# ----------------------------------------------------------------------------
# entry point
# ----------------------------------------------------------------------------

_CACHE = {}
LAST_RESULTS = None


def kernel(**inputs) -> np.ndarray:
    global LAST_RESULTS
    gi = np.asarray(inputs["graph_index"]).astype(np.int64)
    n_wins_needed = len(_build_windows(np.sort(gi), NUM_GRAPHS))
    n_windows = max(WINDOWS_PER_CORE, -(-n_wins_needed // N_CORES))
    if n_windows not in _CACHE:
        _CACHE[n_windows] = build_module(n_windows)
    nc = _CACHE[n_windows]
    wins, in_maps = _prep(inputs, n_windows)
    # a previously-wedged core can fail one run with
    # NRT_EXEC_UNIT_UNRECOVERABLE and reset itself; retry once
    try:
        res = bass_utils.run_bass_kernel_spmd(
            nc, in_maps, core_ids=list(range(N_CORES)))
    except Exception:
        res = bass_utils.run_bass_kernel_spmd(
            nc, in_maps, core_ids=list(range(N_CORES)))
    LAST_RESULTS = res
    return _assemble(wins, res.results, n_windows)


# revision 37
# speedup vs baseline: 1.2039x; 1.2039x over previous
"""AttentionPooling (segment_reduce) Trainium2 kernel.

att = sigmoid([input_rep, final_rep] @ W_lin.T + b_lin)
g   = att * (final_rep @ W_last.T + b_last)
out = segment_sum(g, graph_index, 16384)          # graph_index sorted

Strategy (8 NeuronCores, pure data-parallel, no collectives):
  graph_index is sorted, so a contiguous node range covers a contiguous
  graph range.  Host greedily packs whole graphs into "windows" of
  <= WIN_NODES nodes spanning <= 128 graphs; ~136 windows cover all 500k
  nodes = 8 cores x 17 windows.  Each core gets its windows as a padded
  node stream in feature-major bf16 layout (host pre-transposes + casts,
  so the device only does line-rate DMA and matmuls).

Device pipeline (flat over all subtiles of 128 nodes, paired 2-by-2):
  PE    : per subtile  ones[1,128].T @ biascat  (psum init, row-packed)
          xT_in.T @ WlinT[:128] / xT_fin0.T @ [WlinT[128:256]|WlastT[:128]]
          / xT_fin1.T @ [WlinT[256:]|WlastT[128:]]  -> psum half [128,512]
          (pairs of subtiles share one 2-bank psum tile [128,1024])
  ACT   : sigmoid over BOTH halves' att cols in one 512-col strided op
  DVE   : g = att * val-cols, one 512-col strided op per pair
  PE    : onehot.T @ g accumulated into a window's seg psum, emitted with
          a 4-subtile LAG so the strict-FIFO PE never waits on ACT/DVE.
          onehots are host-precomputed and DMA'd as a 4th bf16 plane
          (device-built onehots cost 280ns/subtile of DVE or 2.4us of
          GpSimd — both measured worse than the extra DMA).
  Per window the [128,256] f32 graph block is copied out + DMA'd; host
  reassembles the window blocks into [16384, 256].
"""

import numpy as np
import ml_dtypes

import concourse.bacc as bacc
import concourse.tile as tile
from concourse import mybir
from concourse import bass_utils
from concourse._compat import with_exitstack

P = 128
HID = 256
WIN_SUB = 29                     # subtiles (128 nodes) per window
WIN_NODES = WIN_SUB * P          # 3712
WINDOWS_PER_CORE = 17
N_CORES = 8
NUM_GRAPHS = 16384
GMAX = P                         # graph span per window
LAG = 6                          # subtiles between linears and seg matmul
                                 # (ACT+DVE chain after psum-stop is ~2us;
                                 # LAG=4 measured 1.16us stalls on seg MMs)

BF16 = mybir.dt.bfloat16
F32 = mybir.dt.float32
F8 = mybir.dt.float8e4
npbf16 = ml_dtypes.bfloat16
npf8 = ml_dtypes.float8_e4m3


# ----------------------------------------------------------------------------
# host-side planning
# ----------------------------------------------------------------------------

def _build_windows(gi: np.ndarray, num_graphs: int):
    """Greedy windows: contiguous whole-graph ranges, graph span <= GMAX,
    node count <= WIN_NODES.  Returns list of (gbase, gcnt, nstart, ncnt)."""
    counts = np.bincount(gi, minlength=num_graphs)
    starts = np.concatenate([[0], np.cumsum(counts)])
    wins = []
    g = 0
    while g < num_graphs:
        base = g
        nodes = 0
        cnt = 0
        while g < num_graphs and cnt < GMAX and nodes + counts[g] <= WIN_NODES:
            nodes += int(counts[g])
            cnt += 1
            g += 1
        if cnt == 0:
            raise ValueError(f"graph {g} has {counts[g]} nodes > {WIN_NODES}")
        wins.append((base, cnt, int(starts[base]), nodes))
    return wins


# ----------------------------------------------------------------------------
# device kernel
# ----------------------------------------------------------------------------

@with_exitstack
def _device_kernel(ctx, tc, out_ap, ins, n_windows):
    nc = tc.nc
    xdr_ap, xf0_ap, xf1_ap, oh_ap, wdr_ap, wlast0_ap, wcat1_ap, \
        biascat_ap, ones_ap = ins

    consts = ctx.enter_context(tc.tile_pool(name="consts", bufs=1))
    xpool = ctx.enter_context(tc.tile_pool(name="x", bufs=3))
    x0pool = ctx.enter_context(tc.tile_pool(name="x0", bufs=1))
    apool = ctx.enter_context(tc.tile_pool(name="act", bufs=3))
    gpool = ctx.enter_context(tc.tile_pool(name="g", bufs=5))
    outpool = ctx.enter_context(tc.tile_pool(name="out", bufs=2))
    ps_pair = ctx.enter_context(tc.tile_pool(name="ps_pair", bufs=3, space="PSUM"))
    ps_seg = ctx.enter_context(tc.tile_pool(name="ps_seg", bufs=2, space="PSUM"))

    # constants, loaded once; ordered so the first matmuls' inputs land first
    ones_t = consts.tile([P, P], BF16)
    nc.sync.dma_start(ones_t[:], ones_ap[:])
    biascat = consts.tile([P, 2 * HID], BF16)
    nc.sync.dma_start(biascat[:], biascat_ap[:])
    wdr = consts.tile([P, 2 * HID], F8)
    nc.sync.dma_start(wdr[:], wdr_ap[:])
    wlast0 = consts.tile([P, HID], BF16)
    nc.sync.dma_start(wlast0[:], wlast0_ap[:])
    wcat1 = consts.tile([P, 2 * HID], BF16)
    nc.sync.dma_start(wcat1[:], wcat1_ap[:])
    wdr3 = wdr.rearrange("p (a b) -> p a b", a=2)

    T = n_windows * WIN_SUB

    # ---- per-window x tiles (window 0 staged in chunks so matmuls can
    # start after ~0.3 MB; later windows are one bulk DMA per plane) ----
    NCHUNK = 3
    win_tiles = [None] * n_windows

    def load_window(w):
        base = w * WIN_NODES
        # xdr plane is window-major [xin8_w | xf08_w] fp8, 2*WIN_NODES cols/window
        db = 2 * base
        if w == 0:
            tiles = []
            for q in range(NCHUNK):
                c0 = q * 4 * P
                cw = 4 * P
                xt = x0pool.tile([P, 2 * cw], F8, tag=f"x0i{q}")
                nc.sync.dma_start(xt[:, 0:cw], xdr_ap[:, db + c0:db + c0 + cw])
                nc.sync.dma_start(
                    xt[:, cw:2 * cw],
                    xdr_ap[:, db + WIN_NODES + c0:db + WIN_NODES + c0 + cw])
                f0 = x0pool.tile([P, cw], BF16, tag=f"x0a{q}")
                nc.sync.dma_start(f0[:], xf0_ap[:, base + c0:base + c0 + cw])
                f1 = x0pool.tile([P, cw], BF16, tag=f"x0b{q}")
                nc.sync.dma_start(f1[:], xf1_ap[:, base + c0:base + c0 + cw])
                oht = x0pool.tile([P, cw], BF16, tag=f"x0o{q}")
                nc.sync.dma_start(oht[:], oh_ap[:, base + c0:base + c0 + cw])
                tiles.append((xt, f0, f1, oht, c0, c0 + cw))
            r0 = NCHUNK * 4 * P
            rw = WIN_NODES - r0
            xr = x0pool.tile([P, 2 * rw], F8, tag="x0ir")
            nc.sync.dma_start(xr[:, 0:rw], xdr_ap[:, db + r0:db + WIN_NODES])
            nc.sync.dma_start(
                xr[:, rw:2 * rw],
                xdr_ap[:, db + WIN_NODES + r0:db + 2 * WIN_NODES])
            f0r = x0pool.tile([P, rw], BF16, tag="x0ar")
            nc.sync.dma_start(f0r[:], xf0_ap[:, base + r0:base + WIN_NODES])
            f1r = x0pool.tile([P, rw], BF16, tag="x0br")
            nc.sync.dma_start(f1r[:], xf1_ap[:, base + r0:base + WIN_NODES])
            ohr = x0pool.tile([P, rw], BF16, tag="x0or")
            nc.sync.dma_start(ohr[:], oh_ap[:, base + r0:base + WIN_NODES])
            tiles.append((xr, f0r, f1r, ohr, r0, WIN_NODES))
            win_tiles[w] = tiles
        else:
            xdr_t = xpool.tile([P, 2 * WIN_NODES], F8, tag="xdr")
            nc.sync.dma_start(xdr_t[:], xdr_ap[:, db:db + 2 * WIN_NODES])
            xf0_t = xpool.tile([P, WIN_NODES], BF16, tag="xf0")
            nc.sync.dma_start(xf0_t[:], xf0_ap[:, base:base + WIN_NODES])
            xf1_t = xpool.tile([P, WIN_NODES], BF16, tag="xf1")
            nc.sync.dma_start(xf1_t[:], xf1_ap[:, base:base + WIN_NODES])
            oh_t = xpool.tile([P, WIN_NODES], BF16, tag="oh")
            nc.sync.dma_start(oh_t[:], oh_ap[:, base:base + WIN_NODES])
            win_tiles[w] = [(xdr_t, xf0_t, xf1_t, oh_t, 0, WIN_NODES)]

    def subtile_x(t):
        """-> (xt, f0, f1, oht, col) for global subtile t."""
        w, s = divmod(t, WIN_SUB)
        c = s * P
        for xt, f0, f1, oht, lo, hi in win_tiles[w]:
            if lo <= c < hi:
                return xt, f0, f1, oht, c - lo
        raise AssertionError

    def emit_mms(ps, off, t):
        """3 accumulating matmuls for subtile t into psum cols [off, off+512).
        The att head (xin + first final half vs WlinT[0:256]) runs as one fp8
        DoubleRow matmul (K=256 virtualized, 2 cols/cycle); the rest is bf16."""
        xdr, f0, f1, _, col = subtile_x(t)
        xdr3 = xdr.rearrange("p (a b) -> p a b", a=2)
        nc.tensor.matmul(ps[:, off:off + HID],
                         lhsT=xdr3[:, :, col:col + P], rhs=wdr3,
                         start=False, stop=False,
                         perf_mode=mybir.MatmulPerfMode.DoubleRow)
        nc.tensor.matmul(ps[:, off + HID:off + 2 * HID],
                         lhsT=f0[:, col:col + P],
                         rhs=wlast0[:, :], start=False, stop=False)
        nc.tensor.matmul(ps[:, off:off + 2 * HID], lhsT=f1[:, col:col + P],
                         rhs=wcat1[:, :], start=False, stop=True)

    def emit_bias(ps, off, row):
        # K=32 all-ones matmul against a biascat whose rows are bias at
        # k%32==0 and zero elsewhere: each of the 4 group members targets a
        # distinct 32-row strip + distinct PSUM bank so they overlap in the
        # array — one 512-col span initializes four subtile psum halves.
        nc.tensor.matmul(ps[:, off:off + 2 * HID],
                         lhsT=ones_t[32 * row:32 * (row + 1), 0:P],
                         rhs=biascat[32 * row:32 * (row + 1), :],
                         start=True, stop=False,
                         tile_position=(32 * row, 0))

    # ---- flat software pipeline over all subtiles ----
    # state rings
    ps_of = {}           # pair index -> psum tile
    g_of = {}            # pair index -> g pair tile [P, 512] (bf16)
    seg_of = {}          # window -> seg psum tile

    n_pairs = T // 2     # T odd: last subtile handled unpaired
    assert T % 2 == 1

    def emit_pair_post(p):
        """ACT sigmoid + DVE mult for pair p (subtiles 2p, 2p+1), each as one
        512-col strided op over both psum halves."""
        ps2 = ps_of[p].rearrange("p (a b) -> p a b", a=2)
        att = apool.tile([P, 2 * HID], BF16, tag="att")
        att2 = att.rearrange("p (a b) -> p a b", a=2)
        nc.scalar.activation(att2, ps2[:, :, 0:HID],
                             mybir.ActivationFunctionType.Sigmoid)
        g_sb = gpool.tile([P, 2 * HID], BF16, tag="g")
        nc.vector.tensor_tensor(
            g_sb.rearrange("p (a b) -> p a b", a=2), att2,
            ps2[:, :, HID:2 * HID], op=mybir.AluOpType.mult)
        g_of[p] = g_sb

    def emit_seg(u):
        """seg matmul for subtile u (called LAG subtiles later)."""
        w, s = divmod(u, WIN_SUB)
        if s == 0:
            # padded to a full 2KB bank so the two in-flight seg windows
            # never share a bank (PE-W + ACT-R same bank is fatal)
            seg_of[w] = ps_seg.tile([P, 2 * HID], F32, tag="seg", name="seg")
        seg = seg_of[w]
        g_sb = g_of[u // 2]
        half = u % 2
        _, _, _, oht, col = subtile_x(u)
        nc.tensor.matmul(seg[:, 0:HID], lhsT=oht[:, col:col + P],
                         rhs=g_sb[:, half * HID:(half + 1) * HID],
                         start=(s == 0), stop=(s == WIN_SUB - 1))
        if s == WIN_SUB - 1:
            out_t = outpool.tile([P, HID], F32)
            nc.scalar.copy(out_t[:], seg[:, 0:HID])
            nc.sync.dma_start(out_ap[w * P:(w + 1) * P, :], out_t[:])

    load_window(0)
    for t in range(T):
        w, s = divmod(t, WIN_SUB)
        if s == 0:
            if w == 0 and n_windows > 1:
                load_window(1)
            if w + 2 < n_windows:
                load_window(w + 2)
        if t % 4 == 0:
            # 4 bias matmuls (2 pair psum tiles) per group
            for u in range(t, min(t + 4, T)):
                if u % 2 == 0:
                    ps_of[u // 2] = ps_pair.tile([P, 4 * HID], F32, tag="ps", name="ps")
                emit_bias(ps_of[u // 2], (u % 2) * 2 * HID, u % 4)
        if t == T - 1:
            # unpaired last subtile (psum + bias already emitted by its group)
            ps = ps_of[t // 2]
            emit_mms(ps, 0, t)
            att1 = apool.tile([P, HID], BF16, tag="att1")
            nc.scalar.activation(att1[:], ps[:, 0:HID],
                                 mybir.ActivationFunctionType.Sigmoid)
            g1 = gpool.tile([P, 2 * HID], BF16, tag="g")
            nc.vector.tensor_tensor(g1[:, 0:HID], att1[:], ps[:, HID:2 * HID],
                                    op=mybir.AluOpType.mult)
            g_of[t // 2] = g1
        else:
            emit_mms(ps_of[t // 2], (t % 2) * 2 * HID, t)
            if t % 2 == 1:
                emit_pair_post(t // 2)
        if t >= LAG:
            emit_seg(t - LAG)
    for u in range(T - LAG, T):
        emit_seg(u)


def build_module(n_windows=WINDOWS_PER_CORE):
    nc = bacc.Bacc("TRN2", debug=False, num_devices=N_CORES)
    nn = n_windows * WIN_NODES
    ins = [
        nc.dram_tensor("xdr", [P, 2 * nn], F8, kind="ExternalInput").ap(),
        nc.dram_tensor("xf0", [P, nn], BF16, kind="ExternalInput").ap(),
        nc.dram_tensor("xf1", [P, nn], BF16, kind="ExternalInput").ap(),
        nc.dram_tensor("oh", [P, nn], BF16, kind="ExternalInput").ap(),
        nc.dram_tensor("wdr", [P, 2 * HID], F8, kind="ExternalInput").ap(),
        nc.dram_tensor("wlast0", [P, HID], BF16, kind="ExternalInput").ap(),
        nc.dram_tensor("wcat1", [P, 2 * HID], BF16, kind="ExternalInput").ap(),
        nc.dram_tensor("biascat", [P, 2 * HID], BF16, kind="ExternalInput").ap(),
        nc.dram_tensor("ones", [P, P], BF16, kind="ExternalInput").ap(),
    ]
    out_ap = nc.dram_tensor("out", [n_windows * P, HID], F32,
                            kind="ExternalOutput").ap()
    with tile.TileContext(nc) as tc:
        _device_kernel(tc, out_ap, ins, n_windows)
    nc.compile()
    return nc


# ----------------------------------------------------------------------------
# host-side data prep
# ----------------------------------------------------------------------------

def _prep(inputs, n_windows):
    gi = np.asarray(inputs["graph_index"]).astype(np.int64)
    x_in = np.asarray(inputs["input_rep"], dtype=np.float32)
    x_fin = np.asarray(inputs["final_rep"], dtype=np.float32)
    W_lin = np.asarray(inputs["W_lin"], dtype=np.float32)
    b_lin = np.asarray(inputs["b_lin"], dtype=np.float32)
    W_last = np.asarray(inputs["W_last"], dtype=np.float32)
    b_last = np.asarray(inputs["b_last"], dtype=np.float32)

    if np.any(np.diff(gi) < 0):
        order = np.argsort(gi, kind="stable")
        gi = gi[order]
        x_in = x_in[order]
        x_fin = x_fin[order]

    wins = _build_windows(gi, NUM_GRAPHS)
    budget = N_CORES * n_windows
    assert len(wins) <= budget, f"{len(wins)} windows > budget {budget}"
    wins = wins + [(NUM_GRAPHS, 0, len(gi), 0)] * (budget - len(wins))

    x_in8 = x_in.astype(npf8)                 # att head via fp8 DoubleRow
    x_f08 = np.ascontiguousarray(x_fin[:, 0:P]).astype(npf8)
    x_fin_b = x_fin.astype(npbf16)

    WlinT = W_lin.T                           # [384, 256] f32
    WlastT = W_last.T.astype(npbf16)          # [256, 256]
    wdr = np.concatenate(
        [WlinT[0:P].astype(npf8), WlinT[P:2 * P].astype(npf8)], axis=1)
    wlast0 = np.ascontiguousarray(WlastT[0:P])
    wcat1 = np.ascontiguousarray(
        np.concatenate([WlinT[2 * P:3 * P].astype(npbf16),
                        WlastT[P:2 * P]], axis=1))
    # bias rows at k%32==0 only: the K=32 all-ones bias matmul reduces the
    # 32-row strip, so a single bias row per strip survives
    biascat = np.zeros((P, 2 * HID), npbf16)
    biascat[0::32, :] = np.concatenate([b_lin, b_last]).astype(npbf16)
    ones_t = np.ones((P, P), npbf16)
    jrange = np.arange(P, dtype=np.int32)

    nn = n_windows * WIN_NODES
    in_maps = []
    for c in range(N_CORES):
        xdr_p = np.zeros((P, 2 * nn), npf8)
        xf0_p = np.zeros((P, nn), npbf16)
        xf1_p = np.zeros((P, nn), npbf16)
        oh_p = np.zeros((P, nn), npbf16)
        for j in range(n_windows):
            gb, gc, ns, ncnt = wins[c * n_windows + j]
            if ncnt == 0:
                continue
            off = j * WIN_NODES
            xdr_p[:, 2 * off:2 * off + ncnt] = x_in8[ns:ns + ncnt].T
            xdr_p[:, 2 * off + WIN_NODES:2 * off + WIN_NODES + ncnt] = \
                x_f08[ns:ns + ncnt].T
            xf0_p[:, off:off + ncnt] = x_fin_b[ns:ns + ncnt, 0:P].T
            xf1_p[:, off:off + ncnt] = x_fin_b[ns:ns + ncnt, P:2 * P].T
            # onehot plane: subtile s = cols [s*128,(s+1)*128); oh[p, s*128+j]
            # = 1 iff node (s*128+p) of the window belongs to local graph j
            flat = np.full((WIN_NODES,), -1, np.int32)
            flat[0:ncnt] = (gi[ns:ns + ncnt] - gb).astype(np.int32)
            idx_mat = flat.reshape(WIN_SUB, P)            # [s, p]
            oh_win = (idx_mat[:, :, None] == jrange).astype(npbf16)  # [s,p,j]
            oh_p[:, off:off + WIN_NODES] = \
                oh_win.transpose(1, 0, 2).reshape(P, WIN_NODES)
        in_maps.append({
            "xdr": xdr_p, "xf0": xf0_p, "xf1": xf1_p, "oh": oh_p,
            "wdr": wdr, "wlast0": wlast0, "wcat1": wcat1,
            "biascat": biascat, "ones": ones_t,
        })
    return wins, in_maps


def _assemble(wins, results, n_windows):
    out = np.zeros((NUM_GRAPHS, HID), np.float32)
    for c in range(N_CORES):
        res = results[c]["out"]
        for j in range(n_windows):
            gb, gc, _, _ = wins[c * n_windows + j]
            if gc == 0:
                continue
            out[gb:gb + gc] = res[j * P:j * P + gc]
    return out


# ----------------------------------------------------------------------------
# entry point
# ----------------------------------------------------------------------------

_CACHE = {}
LAST_RESULTS = None


def kernel(**inputs) -> np.ndarray:
    global LAST_RESULTS
    gi = np.asarray(inputs["graph_index"]).astype(np.int64)
    n_wins_needed = len(_build_windows(np.sort(gi), NUM_GRAPHS))
    n_windows = max(WINDOWS_PER_CORE, -(-n_wins_needed // N_CORES))
    if n_windows not in _CACHE:
        _CACHE[n_windows] = build_module(n_windows)
    nc = _CACHE[n_windows]
    wins, in_maps = _prep(inputs, n_windows)
    # a previously-wedged core can fail one run with
    # NRT_EXEC_UNIT_UNRECOVERABLE and reset itself; retry once
    try:
        res = bass_utils.run_bass_kernel_spmd(
            nc, in_maps, core_ids=list(range(N_CORES)))
    except Exception:
        res = bass_utils.run_bass_kernel_spmd(
            nc, in_maps, core_ids=list(range(N_CORES)))
    LAST_RESULTS = res
    return _assemble(wins, res.results, n_windows)
